# revision 11
# baseline (speedup 1.0000x reference)
"""Bahdanau (additive) attention kernel for Trainium2, 8-core data-parallel.

Math (per batch element b):
    proj[l, o]  = sum_h enc[l, b, h] * w_e[o, h]          (big GEMM, 17 GF/core)
    energy      = tanh(proj + hidden@w_h.T + attn_b)       (bias folded into ACT)
    scores[l]   = sum_o v[o] * energy[o, l]                (v-matmul on PE)
    p           = exp(scores)                              (no max-shift needed;
                                                            |scores| <~ 26 worst case)
    context[o]  = (sum_l p_l * enc[l, b, o]) / sum_l p_l

Sharding: batch B=32 split across 8 cores (4 each); weights replicated.
No collectives.

Data path: encoder tiles are cast fp32->bf16 during the HBM DMA (SWDGE), then
rotated to h-major layout with the DMA xbar transpose (SBUF->SBUF, bf16) so the
PE and DVE never touch the transposes. All matmuls run bf16 at 1 cycle/row.
Per-chunk score/context tail work is deferred into the next chunk's matmul
stream so the PE never waits on the ACT round-trips.
"""

import functools
import os
import sys

import numpy as np

sys.path.insert(0, "/opt/trn_rl_repo")

import concourse.bass as bass  # noqa: E402
import concourse.tile as tile  # noqa: E402
from concourse import bacc, mybir  # noqa: E402
from concourse.bass import ts  # noqa: E402
from concourse.masks import make_identity  # noqa: E402

# This container's slim axon client lacks the NTFF profile hook module that
# run_bass_kernel_spmd's trace path imports; give it a graceful no-op fallback
# so a BASS_TRACE env var doesn't crash the run.
try:
    from antenv import axon_hooks as _axon_hooks  # noqa: F401
except Exception:
    import types as _types

    _stub = _types.ModuleType("antenv.axon_hooks")
    _stub.get_axon_ntff_profile_hook = lambda: None
    sys.modules["antenv.axon_hooks"] = _stub

B, L, H = 32, 2048, 1024
N_CORES = 8
B_LOC = B // N_CORES

F32 = mybir.dt.float32
F32R = mybir.dt.float32r
BF16 = mybir.dt.bfloat16
AF = mybir.ActivationFunctionType

LAST_RESULTS = None  # BassKernelResults of the most recent hw run (for test.py)


def build_attn_kernel(tc, out_ap, ins, b_loc=B_LOC, l_total=L, n_repeat=1):
    """Trace the per-core kernel into TileContext tc.

    ins: dict of DRAM APs keyed hidden/encoder_outputs/attn_w/attn_b/v
    out_ap: DRAM AP [b_loc, H]
    """
    nc = tc.nc
    assert H == 1024
    HT = H // 128  # 8 h-tiles
    OT = H // 128  # 8 o-tiles
    CH = 512  # l-chunk (moving free dim of the main matmuls)
    n_ch = l_total // CH
    LT = CH // 128  # l-tiles per chunk

    enc = ins["encoder_outputs"]  # [l_total, b_loc, H]

    from contextlib import ExitStack

    with ExitStack() as ctx:
        const = ctx.enter_context(tc.tile_pool(name="const", bufs=1))
        wnat_pool = ctx.enter_context(tc.tile_pool(name="wnat", bufs=2))
        nat_pool = ctx.enter_context(tc.tile_pool(name="nat", bufs=3))
        encT_pool = ctx.enter_context(tc.tile_pool(name="encT", bufs=3))
        eng_pool = ctx.enter_context(tc.tile_pool(name="eng", bufs=4))
        small = ctx.enter_context(tc.tile_pool(name="small", bufs=2))
        p_pool = ctx.enter_context(tc.tile_pool(name="pp", bufs=8))
        psum_mm = ctx.enter_context(tc.tile_pool(name="psmm", bufs=3, space="PSUM"))
        psum_tr = ctx.enter_context(tc.tile_pool(name="pstr", bufs=2, space="PSUM"))
        psum_sm = ctx.enter_context(tc.tile_pool(name="pssm", bufs=3, space="PSUM"))

        for _rep in range(n_repeat):
            _build_once(
                nc, tc, out_ap, ins, b_loc, l_total,
                const, wnat_pool, nat_pool, encT_pool, eng_pool, small, p_pool,
                psum_mm, psum_tr, psum_sm,
            )


def _build_once(
    nc, tc, out_ap, ins, b_loc, l_total,
    const, wnat_pool, nat_pool, encT_pool, eng_pool, small, p_pool,
    psum_mm, psum_tr, psum_sm,
):
    HT = H // 128
    OT = H // 128
    CH = 512
    n_ch = l_total // CH
    LT = CH // 128
    enc = ins["encoder_outputs"]
    if True:
        # ---------------- constants ----------------
        idb = const.tile([b_loc, b_loc], F32, name="idb", tag="idb")
        make_identity(nc, idb)
        id1 = const.tile([1, 1], F32, name="id1", tag="id1")
        make_identity(nc, id1)

        # attn_b, v as [128, 8] column-per-o-tile layouts
        attn_b_sb = const.tile([128, OT], F32, name="attn_b_sb", tag="attn_b_sb")
        nc.sync.dma_start(attn_b_sb, ins["attn_b"].rearrange("(t p) -> p t", p=128))
        v_f32 = const.tile([128, OT], F32, name="v_f32", tag="v_f32")
        nc.sync.dma_start(v_f32, ins["v"].rearrange("(t p) -> p t", p=128))
        v_bf = const.tile([128, OT], BF16, name="v_bf", tag="v_bf")
        nc.vector.tensor_copy(v_bf, v_f32)

        # ---------------- weights: cast to bf16, xbar-transpose ----------------
        # w_T3: [128(part c%128), 16(c tile), 1024(o)] bf16 ; w_T[c, o] = attn_w[o, c]
        w_T3 = const.tile([128, 2 * HT, H], BF16, name="w_T3", tag="w_T3")
        for oi in range(OT):
            wnat = wnat_pool.tile([128, 2 * H], BF16, name="wnat", tag="wnat")
            nc.gpsimd.dma_start(wnat, ins["attn_w"][ts(oi, 128), :])  # fp32->bf16
            nc.sync.dma_start(w_T3[:, :, ts(oi, 128)], wnat, transpose=True)

        # ---------------- hidden transpose + hidden_proj + bias ----------------
        hid_sb = const.tile([b_loc, H], F32, name="hid_sb", tag="hid_sb")
        nc.sync.dma_start(hid_sb, ins["hidden"])
        hT = const.tile([128, HT * b_loc], BF16, name="hT", tag="hT")
        for hi in range(HT):
            htr_ps = psum_tr.tile([128, b_loc], F32, name="htr_ps", tag="tr")
            nc.tensor.transpose(htr_ps, hid_sb[:, ts(hi, 128)], idb)
            nc.vector.tensor_copy(hT[:, ts(hi, b_loc)], htr_ps)

        # bias_sb[:, oi*b_loc + b] = hidden_proj[b, oi-tile] + attn_b[oi-tile]
        bias_sb = const.tile([128, OT * b_loc], F32, name="bias_sb", tag="bias_sb")
        for oi in range(OT):
            hp_ps = psum_tr.tile([128, b_loc], F32, name="hp_ps", tag="tr")
            for hi in range(HT):
                nc.tensor.matmul(
                    hp_ps,
                    w_T3[:, hi, ts(oi, 128)],
                    hT[:, ts(hi, b_loc)],
                    start=(hi == 0),
                    stop=(hi == HT - 1),
                )
            nc.scalar.activation(
                bias_sb[:, ts(oi, b_loc)],
                hp_ps,
                AF.Identity,
                bias=attn_b_sb[:, oi : oi + 1],
                scale=1.0,
            )

        # ---------------- main loop ----------------
        for b in range(b_loc):
            ctx_sb = small.tile([1, H], F32, name="ctx_sb", tag="ctx_sb")
            nc.gpsimd.memset(ctx_sb, 0.0)
            denom_part = small.tile([128, 1], F32, name="denom_part", tag="den")
            nc.gpsimd.memset(denom_part, 0.0)

            pending_tail = None
            for c in range(n_ch):
                l0 = c * CH
                # one cast DMA per chunk: nat_all[l_lo, lt, h] = enc[l0+lt*128+l_lo, b, h]
                nat_all = nat_pool.tile([128, LT, H], BF16, name="nat_all", tag="nat")
                nc.gpsimd.dma_start(
                    nat_all,
                    enc[l0 : l0 + CH, b, :].rearrange("(lt p) h -> p lt h", p=128),
                )
                # xbar-transpose: encT_all[h_lo, lt, hi, l_lo] = nat_all[l_lo, lt, hi*128+h_lo]
                encT_all = encT_pool.tile(
                    [128, LT, HT, 128], BF16, name="encT_all", tag="encT"
                )
                for lt in range(LT):
                    nc.sync.dma_start(encT_all[:, lt], nat_all[:, lt], transpose=True)

                # main GEMM; tanh; v-dot one group behind the matmuls
                sc_ps = psum_sm.tile([1, CH], F32, name="sc_ps", tag="sm")
                engs = [None] * OT
                for oi in range(OT):
                    mm_ps = psum_mm.tile([128, CH], F32, name="mm_ps", tag="mm")
                    for hi in range(HT):
                        nc.tensor.matmul(
                            mm_ps,
                            w_T3[:, HT + hi, ts(oi, 128)],
                            encT_all[:, :, hi, :],
                            start=(hi == 0),
                            stop=(hi == HT - 1),
                        )
                    if oi == 0 and pending_tail is not None:
                        pending_tail()
                        pending_tail = None
                    eng = eng_pool.tile([128, CH], BF16, name="eng", tag="eng")
                    nc.scalar.activation(
                        eng,
                        mm_ps,
                        AF.Tanh,
                        bias=bias_sb[:, oi * b_loc + b : oi * b_loc + b + 1],
                        scale=1.0,
                    )
                    engs[oi] = eng
                    if oi > 0:
                        nc.tensor.matmul(
                            sc_ps,
                            v_bf[:, oi - 1 : oi],
                            engs[oi - 1],
                            start=(oi == 1),
                            stop=False,
                        )

                def make_tail(sc_ps=sc_ps, engs=engs, nat_all=nat_all):
                    def tail():
                        # last v-matmul of the chunk (its tanh finished long ago)
                        nc.tensor.matmul(
                            sc_ps,
                            v_bf[:, OT - 1 : OT],
                            engs[OT - 1],
                            start=False,
                            stop=True,
                        )
                        sc_sb = small.tile([1, CH], F32, name="sc_sb", tag="sc_sb")
                        nc.scalar.copy(sc_sb, sc_ps)
                        p_sbs = []
                        for lt in range(LT):
                            pt_ps = psum_sm.tile([128, 1], F32, name="pt_ps", tag="sm")
                            nc.tensor.transpose(pt_ps, sc_sb[:, ts(lt, 128)], id1)
                            p_sb = p_pool.tile([128, 1], BF16, name="p_sb", tag="p")
                            nc.scalar.activation(p_sb, pt_ps, AF.Exp)
                            nc.vector.tensor_add(denom_part, denom_part, p_sb)
                            p_sbs.append(p_sb)
                        for half in range(2):
                            cx_ps = psum_sm.tile([1, 512], F32, name="cx_ps", tag="sm")
                            for lt in range(LT):
                                nc.tensor.matmul(
                                    cx_ps,
                                    p_sbs[lt],
                                    nat_all[:, lt, ts(half, 512)],
                                    start=(lt == 0),
                                    stop=(lt == LT - 1),
                                )
                            nc.vector.tensor_add(
                                ctx_sb[:, ts(half, 512)],
                                ctx_sb[:, ts(half, 512)],
                                cx_ps,
                            )

                    return tail

                pending_tail = make_tail()

            pending_tail()

            # normalize: context / sum(p)
            den_sb = small.tile([1, 1], F32, name="den_sb", tag="den_sb")
            nc.gpsimd.tensor_reduce(
                den_sb, denom_part, mybir.AxisListType.C, mybir.AluOpType.add
            )
            recip = small.tile([1, 1], F32, name="recip", tag="recip")
            nc.vector.reciprocal(recip, den_sb)
            outb = small.tile([1, H], F32, name="outb", tag="outb")
            nc.scalar.activation(outb, ctx_sb, AF.Copy, bias=0.0, scale=recip)
            nc.sync.dma_start(out_ap[b : b + 1, :], outb)


def build_bass(b_loc=B_LOC, l_total=L, enable_asserts=False, n_repeat=1):
    """Build + schedule + compile the Bass module. Returns (nc, out_name)."""
    nc = bacc.Bacc(
        "TRN2",
        target_bir_lowering=False,
        debug=False,
        enable_asserts=enable_asserts,
        num_devices=N_CORES,
    )
    ins = {
        "hidden": nc.dram_tensor("hidden", [b_loc, H], F32, kind="ExternalInput").ap(),
        "encoder_outputs": nc.dram_tensor(
            "encoder_outputs", [l_total, b_loc, H], F32, kind="ExternalInput"
        ).ap(),
        "attn_w": nc.dram_tensor("attn_w", [H, 2 * H], F32, kind="ExternalInput").ap(),
        "attn_b": nc.dram_tensor("attn_b", [H], F32, kind="ExternalInput").ap(),
        "v": nc.dram_tensor("v", [H], F32, kind="ExternalInput").ap(),
    }
    out = nc.dram_tensor("ctx_out", [b_loc, H], F32, kind="ExternalOutput").ap()
    with tile.TileContext(nc) as tc:
        build_attn_kernel(tc, out, ins, b_loc=b_loc, l_total=l_total,
                          n_repeat=n_repeat)
    nc.compile()
    return nc, "ctx_out"


@functools.cache
def _built():
    return build_bass()


def kernel(hidden, encoder_outputs, attn_w, attn_b, v):
    """Full-input entry point: shard over batch, run 8 cores, gather."""
    global LAST_RESULTS
    from concourse.bass_utils import run_bass_kernel_spmd

    hidden = np.ascontiguousarray(np.asarray(hidden, dtype=np.float32))
    encoder_outputs = np.ascontiguousarray(
        np.asarray(encoder_outputs, dtype=np.float32)
    )
    attn_w = np.ascontiguousarray(np.asarray(attn_w, dtype=np.float32))
    attn_b = np.ascontiguousarray(np.asarray(attn_b, dtype=np.float32))
    v = np.ascontiguousarray(np.asarray(v, dtype=np.float32))

    nc, out_name = _built()
    in_maps = []
    for c in range(N_CORES):
        bs = slice(c * B_LOC, (c + 1) * B_LOC)
        in_maps.append(
            {
                "hidden": np.ascontiguousarray(hidden[bs]),
                "encoder_outputs": np.ascontiguousarray(encoder_outputs[:, bs, :]),
                "attn_w": attn_w,
                "attn_b": attn_b,
                "v": v,
            }
        )
    res = run_bass_kernel_spmd(
        nc,
        in_maps,
        core_ids=list(range(N_CORES)),
        trace=bool(os.environ.get("BASS_TRACE")),
    )
    LAST_RESULTS = res
    out = np.concatenate([res.results[c][out_name] for c in range(N_CORES)], axis=0)
    return out[None, :, :].astype(np.float32)


# revision 12
# speedup vs baseline: 1.6530x; 1.6530x over previous
"""Bahdanau (additive) attention kernel for Trainium2, 8-core data-parallel.

Math (per batch element b):
    proj[l, o]  = sum_h enc[l, b, h] * w_e[o, h]          (big GEMM, 17 GF/core)
    energy      = tanh(proj + hidden@w_h.T + attn_b)       (bias folded into ACT)
    scores[l]   = sum_o v[o] * energy[o, l]                (v-matmul on PE)
    p           = exp(scores)                              (no max-shift needed;
                                                            |scores| <~ 26 worst case)
    context[o]  = (sum_l p_l * enc[l, b, o]) / sum_l p_l

Sharding: batch B=32 split across 8 cores (4 each); weights replicated.
No collectives.

Data path: encoder tiles are cast fp32->bf16 during the HBM DMA (SWDGE), then
rotated to h-major layout with the DMA xbar transpose (SBUF->SBUF, bf16) so the
PE and DVE never touch the transposes. All matmuls run bf16 at 1 cycle/row.
Per-chunk score/context tail work is deferred into the next chunk's matmul
stream so the PE never waits on the ACT round-trips.
"""

import functools
import os
import sys

import numpy as np

sys.path.insert(0, "/opt/trn_rl_repo")

import concourse.bass as bass  # noqa: E402
import concourse.tile as tile  # noqa: E402
from concourse import bacc, mybir  # noqa: E402
from concourse.bass import ts  # noqa: E402
from concourse.masks import make_identity  # noqa: E402

# This container's slim axon client lacks the NTFF profile hook module that
# run_bass_kernel_spmd's trace path imports; give it a graceful no-op fallback
# so a BASS_TRACE env var doesn't crash the run.
try:
    from antenv import axon_hooks as _axon_hooks  # noqa: F401
except Exception:
    import types as _types

    _stub = _types.ModuleType("antenv.axon_hooks")
    _stub.get_axon_ntff_profile_hook = lambda: None
    sys.modules["antenv.axon_hooks"] = _stub

B, L, H = 32, 2048, 1024
N_CORES = 8
B_LOC = B // N_CORES

F32 = mybir.dt.float32
F32R = mybir.dt.float32r
BF16 = mybir.dt.bfloat16
AF = mybir.ActivationFunctionType

LAST_RESULTS = None  # BassKernelResults of the most recent hw run (for test.py)


def build_attn_kernel(tc, out_ap, ins, b_loc=B_LOC, l_total=L, n_repeat=1):
    """Trace the per-core kernel into TileContext tc.

    ins: dict of DRAM APs keyed hidden/encoder_outputs/attn_w/attn_b/v
    out_ap: DRAM AP [b_loc, H]
    """
    nc = tc.nc
    assert H == 1024
    HT = H // 128  # 8 h-tiles
    OT = H // 128  # 8 o-tiles
    CH = 512  # l-chunk (moving free dim of the main matmuls)
    n_ch = l_total // CH
    LT = CH // 128  # l-tiles per chunk

    enc = ins["encoder_outputs"]  # [l_total, b_loc, H]

    from contextlib import ExitStack

    with ExitStack() as ctx:
        const = ctx.enter_context(tc.tile_pool(name="const", bufs=1))
        wnat_pool = ctx.enter_context(tc.tile_pool(name="wnat", bufs=2))
        nat_pool = ctx.enter_context(tc.tile_pool(name="nat", bufs=3))
        encT_pool = ctx.enter_context(tc.tile_pool(name="encT", bufs=3))
        eng_pool = ctx.enter_context(tc.tile_pool(name="eng", bufs=4))
        small = ctx.enter_context(tc.tile_pool(name="small", bufs=2))
        p_pool = ctx.enter_context(tc.tile_pool(name="pp", bufs=8))
        psum_mm = ctx.enter_context(tc.tile_pool(name="psmm", bufs=3, space="PSUM"))
        psum_tr = ctx.enter_context(tc.tile_pool(name="pstr", bufs=2, space="PSUM"))
        psum_sm = ctx.enter_context(tc.tile_pool(name="pssm", bufs=3, space="PSUM"))

        for _rep in range(n_repeat):
            _build_once(
                nc, tc, out_ap, ins, b_loc, l_total,
                const, wnat_pool, nat_pool, encT_pool, eng_pool, small, p_pool,
                psum_mm, psum_tr, psum_sm,
            )


def _build_once(
    nc, tc, out_ap, ins, b_loc, l_total,
    const, wnat_pool, nat_pool, encT_pool, eng_pool, small, p_pool,
    psum_mm, psum_tr, psum_sm,
):
    HT = H // 128
    OT = H // 128
    CH = 512
    n_ch = l_total // CH
    LT = CH // 128
    enc = ins["encoder_outputs"]
    if True:
        # ---------------- constants ----------------
        idb = const.tile([b_loc, b_loc], F32, name="idb", tag="idb")
        make_identity(nc, idb)
        id1 = const.tile([1, 1], F32, name="id1", tag="id1")
        make_identity(nc, id1)

        # attn_b, v as [128, 8] column-per-o-tile layouts
        attn_b_sb = const.tile([128, OT], F32, name="attn_b_sb", tag="attn_b_sb")
        nc.sync.dma_start(attn_b_sb, ins["attn_b"].rearrange("(t p) -> p t", p=128))
        v_f32 = const.tile([128, OT], F32, name="v_f32", tag="v_f32")
        nc.sync.dma_start(v_f32, ins["v"].rearrange("(t p) -> p t", p=128))
        v_bf = const.tile([128, OT], BF16, name="v_bf", tag="v_bf")
        nc.vector.tensor_copy(v_bf, v_f32)

        # ---------------- weights: cast to bf16, xbar-transpose ----------------
        # w_T3: [128(part c%128), 16(c tile), 1024(o)] bf16 ; w_T[c, o] = attn_w[o, c]
        w_T3 = const.tile([128, 2 * HT, H], BF16, name="w_T3", tag="w_T3")
        for oi in range(OT):
            wnat = wnat_pool.tile([128, 2 * H], BF16, name="wnat", tag="wnat")
            nc.gpsimd.dma_start(wnat, ins["attn_w"][ts(oi, 128), :])  # fp32->bf16
            nc.sync.dma_start(w_T3[:, :, ts(oi, 128)], wnat, transpose=True)

        # ---------------- hidden transpose + hidden_proj + bias ----------------
        hid_sb = const.tile([b_loc, H], F32, name="hid_sb", tag="hid_sb")
        nc.sync.dma_start(hid_sb, ins["hidden"])
        hT = const.tile([128, HT * b_loc], BF16, name="hT", tag="hT")
        for hi in range(HT):
            htr_ps = psum_tr.tile([128, b_loc], F32, name="htr_ps", tag="tr")
            nc.tensor.transpose(htr_ps, hid_sb[:, ts(hi, 128)], idb)
            nc.vector.tensor_copy(hT[:, ts(hi, b_loc)], htr_ps)

        # bias_sb[:, oi*b_loc + b] = hidden_proj[b, oi-tile] + attn_b[oi-tile]
        bias_sb = const.tile([128, OT * b_loc], F32, name="bias_sb", tag="bias_sb")
        for oi in range(OT):
            hp_ps = psum_tr.tile([128, b_loc], F32, name="hp_ps", tag="tr")
            for hi in range(HT):
                nc.tensor.matmul(
                    hp_ps,
                    w_T3[:, hi, ts(oi, 128)],
                    hT[:, ts(hi, b_loc)],
                    start=(hi == 0),
                    stop=(hi == HT - 1),
                )
            nc.scalar.activation(
                bias_sb[:, ts(oi, b_loc)],
                hp_ps,
                AF.Identity,
                bias=attn_b_sb[:, oi : oi + 1],
                scale=1.0,
            )

        # ---------------- main loop ----------------
        for b in range(b_loc):
            ctx_sb = small.tile([1, H], F32, name="ctx_sb", tag="ctx_sb")
            nc.gpsimd.memset(ctx_sb, 0.0)
            denom_part = small.tile([128, 1], F32, name="denom_part", tag="den")
            nc.gpsimd.memset(denom_part, 0.0)

            pending_tail = None
            for c in range(n_ch):
                l0 = c * CH
                # one cast DMA per chunk: nat_all[l_lo, lt, h] = enc[l0+lt*128+l_lo, b, h]
                nat_all = nat_pool.tile([128, LT, H], BF16, name="nat_all", tag="nat")
                if os.environ.get("K_CAST") == "dve":
                    nat_f32 = nat_pool.tile(
                        [128, LT, H], F32, name="nat_f32", tag="natf"
                    )
                    nc.sync.dma_start(
                        nat_f32,
                        enc[l0 : l0 + CH, b, :].rearrange("(lt p) h -> p lt h", p=128),
                    )
                    nc.vector.tensor_copy(nat_all, nat_f32)
                else:
                    nc.gpsimd.dma_start(
                        nat_all,
                        enc[l0 : l0 + CH, b, :].rearrange("(lt p) h -> p lt h", p=128),
                    )
                # xbar-transpose: encT_all[h_lo, lt, hi, l_lo] = nat_all[l_lo, lt, hi*128+h_lo]
                encT_all = encT_pool.tile(
                    [128, LT, HT, 128], BF16, name="encT_all", tag="encT"
                )
                for lt in range(LT):
                    nc.sync.dma_start(encT_all[:, lt], nat_all[:, lt], transpose=True)

                # main GEMM; tanh; v-dot one group behind the matmuls
                sc_ps = psum_sm.tile([1, CH], F32, name="sc_ps", tag="sm")
                engs = [None] * OT
                for oi in range(OT):
                    mm_ps = psum_mm.tile([128, CH], F32, name="mm_ps", tag="mm")
                    for hi in range(HT):
                        nc.tensor.matmul(
                            mm_ps,
                            w_T3[:, HT + hi, ts(oi, 128)],
                            encT_all[:, :, hi, :],
                            start=(hi == 0),
                            stop=(hi == HT - 1),
                        )
                    if oi == 0 and pending_tail is not None:
                        pending_tail()
                        pending_tail = None
                    eng = eng_pool.tile([128, CH], BF16, name="eng", tag="eng")
                    nc.scalar.activation(
                        eng,
                        mm_ps,
                        AF.Tanh,
                        bias=bias_sb[:, oi * b_loc + b : oi * b_loc + b + 1],
                        scale=1.0,
                    )
                    engs[oi] = eng
                    if oi > 0:
                        nc.tensor.matmul(
                            sc_ps,
                            v_bf[:, oi - 1 : oi],
                            engs[oi - 1],
                            start=(oi == 1),
                            stop=False,
                        )

                def make_tail(sc_ps=sc_ps, engs=engs, nat_all=nat_all):
                    def tail():
                        # last v-matmul of the chunk (its tanh finished long ago)
                        nc.tensor.matmul(
                            sc_ps,
                            v_bf[:, OT - 1 : OT],
                            engs[OT - 1],
                            start=False,
                            stop=True,
                        )
                        sc_sb = small.tile([1, CH], F32, name="sc_sb", tag="sc_sb")
                        nc.scalar.copy(sc_sb, sc_ps)
                        p_sbs = []
                        for lt in range(LT):
                            pt_ps = psum_sm.tile([128, 1], F32, name="pt_ps", tag="sm")
                            nc.tensor.transpose(pt_ps, sc_sb[:, ts(lt, 128)], id1)
                            p_sb = p_pool.tile([128, 1], BF16, name="p_sb", tag="p")
                            nc.scalar.activation(p_sb, pt_ps, AF.Exp)
                            nc.vector.tensor_add(denom_part, denom_part, p_sb)
                            p_sbs.append(p_sb)
                        for half in range(2):
                            cx_ps = psum_sm.tile([1, 512], F32, name="cx_ps", tag="sm")
                            for lt in range(LT):
                                nc.tensor.matmul(
                                    cx_ps,
                                    p_sbs[lt],
                                    nat_all[:, lt, ts(half, 512)],
                                    start=(lt == 0),
                                    stop=(lt == LT - 1),
                                )
                            nc.vector.tensor_add(
                                ctx_sb[:, ts(half, 512)],
                                ctx_sb[:, ts(half, 512)],
                                cx_ps,
                            )

                    return tail

                pending_tail = make_tail()

            pending_tail()

            # normalize: context / sum(p)
            den_sb = small.tile([1, 1], F32, name="den_sb", tag="den_sb")
            nc.gpsimd.tensor_reduce(
                den_sb, denom_part, mybir.AxisListType.C, mybir.AluOpType.add
            )
            recip = small.tile([1, 1], F32, name="recip", tag="recip")
            nc.vector.reciprocal(recip, den_sb)
            outb = small.tile([1, H], F32, name="outb", tag="outb")
            nc.scalar.activation(outb, ctx_sb, AF.Copy, bias=0.0, scale=recip)
            nc.sync.dma_start(out_ap[b : b + 1, :], outb)


def build_bass(b_loc=B_LOC, l_total=L, enable_asserts=False, n_repeat=1):
    """Build + schedule + compile the Bass module. Returns (nc, out_name)."""
    nc = bacc.Bacc(
        "TRN2",
        target_bir_lowering=False,
        debug=False,
        enable_asserts=enable_asserts,
        num_devices=N_CORES,
    )
    ins = {
        "hidden": nc.dram_tensor("hidden", [b_loc, H], F32, kind="ExternalInput").ap(),
        "encoder_outputs": nc.dram_tensor(
            "encoder_outputs", [l_total, b_loc, H], F32, kind="ExternalInput"
        ).ap(),
        "attn_w": nc.dram_tensor("attn_w", [H, 2 * H], F32, kind="ExternalInput").ap(),
        "attn_b": nc.dram_tensor("attn_b", [H], F32, kind="ExternalInput").ap(),
        "v": nc.dram_tensor("v", [H], F32, kind="ExternalInput").ap(),
    }
    out = nc.dram_tensor("ctx_out", [b_loc, H], F32, kind="ExternalOutput").ap()
    with tile.TileContext(nc) as tc:
        build_attn_kernel(tc, out, ins, b_loc=b_loc, l_total=l_total,
                          n_repeat=n_repeat)
    nc.compile()
    return nc, "ctx_out"


@functools.cache
def _built():
    return build_bass()


def kernel(hidden, encoder_outputs, attn_w, attn_b, v):
    """Full-input entry point: shard over batch, run 8 cores, gather."""
    global LAST_RESULTS
    from concourse.bass_utils import run_bass_kernel_spmd

    hidden = np.ascontiguousarray(np.asarray(hidden, dtype=np.float32))
    encoder_outputs = np.ascontiguousarray(
        np.asarray(encoder_outputs, dtype=np.float32)
    )
    attn_w = np.ascontiguousarray(np.asarray(attn_w, dtype=np.float32))
    attn_b = np.ascontiguousarray(np.asarray(attn_b, dtype=np.float32))
    v = np.ascontiguousarray(np.asarray(v, dtype=np.float32))

    nc, out_name = _built()
    in_maps = []
    for c in range(N_CORES):
        bs = slice(c * B_LOC, (c + 1) * B_LOC)
        in_maps.append(
            {
                "hidden": np.ascontiguousarray(hidden[bs]),
                "encoder_outputs": np.ascontiguousarray(encoder_outputs[:, bs, :]),
                "attn_w": attn_w,
                "attn_b": attn_b,
                "v": v,
            }
        )
    res = run_bass_kernel_spmd(
        nc,
        in_maps,
        core_ids=list(range(N_CORES)),
        trace=bool(os.environ.get("BASS_TRACE")),
    )
    LAST_RESULTS = res
    out = np.concatenate([res.results[c][out_name] for c in range(N_CORES)], axis=0)
    return out[None, :, :].astype(np.float32)
